# revision 32
# baseline (speedup 1.0000x reference)
"""Distributed causal multi-head attention for 8 TRN2 NeuronCores (v3).

Sharding: 24 (batch, head) units -> 8 cores x 3 heads (tensor parallel over
heads, data parallel over batch; cores 0-3 = batch 0, cores 4-7 = batch 1).

Structure (all phases interleaved per 512-query chunk):
- x is shipped pre-transposed bf16 [768, 2048] (pure layout/transport; the
  f32->bf16 rounding equals the on-device cast), all weights/masks bf16
  (mask multiply on device).
- Per chunk qc: project q/k/v for token chunk qc (B), v to natural layout
  (C), causal softmax-attention over k-tiles 0..4qc+3 (D), per-core partial
  output projection z_local @ Wo_local (E), then a pipelined 4-core
  ReduceScatter and bias add for that chunk's output rows.
- PE warm-up matmuls + a dummy collective at t=0 hide the HAM clock ramp and
  the first-collective rendezvous cost.
- Head pair (0,1) is packed into one 128-contraction matmul for E via a
  SBUF->SBUF partition-shift DMA of head 1's z.
"""
import sys
import math
import numpy as np

sys.path.insert(0, "/opt/trn_rl_repo")

D_MODEL, N_HEADS, D_HEAD = 768, 12, 64
BATCH, SEQ = 2, 2048
HPC = 3              # heads per core
GROUP = 4            # cores per batch group
N_CORES = 8
CHQ = 512            # query-chunk width
KT = 128             # key-tile height
NQC = SEQ // CHQ     # 4
NKQ = CHQ // KT      # k-tiles per chunk: 4
NT = SEQ // KT       # 16 token tiles
ND = D_MODEL // 128  # 6 contraction chunks
DC = 384             # out-proj column chunk (2 per 768)
SCALE = 1.0 / math.sqrt(D_HEAD)

# qkv packing into 5 [128, SEQ] tiles; slot s -> tile s//2, base 64*(s%2).
# Order chosen so q_h and k_h always share the same base partition.
SLOTS = [("q", 0), ("q", 1), ("k", 0), ("k", 1), ("q", 2),
         ("v", 0), ("k", 2), ("v", 1), ("v", 2)]
SLOT = {key: s for s, key in enumerate(SLOTS)}

_BUILT = None


def _build():
    import concourse.bass as bass
    import concourse.bacc as bacc
    import concourse.mybir as mybir
    import concourse.tile as tile
    from concourse.masks import make_identity

    f32 = mybir.dt.float32
    bf16 = mybir.dt.bfloat16
    FT = mybir.ActivationFunctionType

    nc = bacc.Bacc("TRN2", target_bir_lowering=False, debug=False,
                   num_devices=N_CORES)

    xT_d = nc.dram_tensor("xT", [D_MODEL, SEQ], bf16, kind="ExternalInput")
    wqkv_d = nc.dram_tensor("wqkv", [D_MODEL, 576], bf16, kind="ExternalInput")
    mqkv_d = nc.dram_tensor("mqkv", [D_MODEL, 576], bf16, kind="ExternalInput")
    bqkv_d = nc.dram_tensor("bqkv", [5, 128], f32, kind="ExternalInput")
    wo_d = nc.dram_tensor("wo", [HPC * D_HEAD, D_MODEL], bf16, kind="ExternalInput")
    mo_d = nc.dram_tensor("mo", [HPC * D_HEAD, D_MODEL], bf16, kind="ExternalInput")
    bo_d = nc.dram_tensor("bo", [1, D_MODEL], f32, kind="ExternalInput")
    out_d = nc.dram_tensor("out", [CHQ, D_MODEL], f32, kind="ExternalOutput")

    with tile.TileContext(nc) as tc:
        with tc.tile_pool(name="const", bufs=1) as constp, \
             tc.tile_pool(name="dram", bufs=1, space="DRAM") as dramp:

            # ---- constants ----
            ident32 = constp.tile([128, 128], f32, tag="id32")
            make_identity(nc, ident32[:])
            ident_r = constp.tile([128, 128], bf16, tag="idr")
            nc.vector.tensor_copy(ident_r[:], ident32[:])
            # tri[p, f] = 1.0 if f >= p else 0.0  (inclusive-diagonal upper tri)
            tri32 = constp.tile([KT, KT], f32, tag="tri32")
            nc.gpsimd.memset(tri32[:], 1.0)
            nc.gpsimd.affine_select(
                out=tri32[:], in_=tri32[:], compare_op=mybir.AluOpType.is_ge,
                fill=0.0, base=0, channel_multiplier=-1, pattern=[[1, KT]])
            tri = constp.tile([KT, KT], bf16, tag="tri")
            nc.vector.tensor_copy(tri[:], tri32[:])
            ones3 = constp.tile([128, HPC], bf16, tag="ones3")
            nc.vector.memset(ones3[:], 1.0)
            ones1 = constp.tile([1, 128], f32, tag="ones1")
            nc.vector.memset(ones1[:], 1.0)
            ones1_r = constp.tile([1, 128], bf16, tag="ones1_r")
            nc.vector.tensor_copy(ones1_r[:], ones1[:])
            ones128b = constp.tile([128, 128], bf16, tag="ones128b")
            nc.vector.memset(ones128b[:], 1.0)
            bias_sb = constp.tile([128, 5], f32, tag="bias")
            warm1 = constp.tile([1, 128], f32, tag="warm1")
            nc.scalar.activation(warm1[:], ones1[:], FT.Exp, scale=0.1)

            # bias_bc[p, :] = b_O for every partition p (for post-RS add)
            bias_bc = constp.tile([128, D_MODEL], f32, tag="bias_bc")
            with tc.tile_pool(name="bldr", bufs=1) as bldrp, \
                 tc.tile_pool(name="psbb", bufs=2, space="PSUM") as psbb:
                bo32 = bldrp.tile([1, D_MODEL], f32, tag="bo32")
                nc.gpsimd.dma_start(out=bo32[:], in_=bo_d[:])
                bo_r = bldrp.tile([1, D_MODEL], bf16, tag="bo_r")
                nc.vector.tensor_copy(bo_r[:], bo32[:])
                for dc in range(2):
                    pb = psbb.tile([128, DC], f32, tag="pb")
                    nc.tensor.matmul(pb[:], ones1_r[:],
                                     bo_r[:, DC * dc:DC * (dc + 1)],
                                     start=True, stop=True)
                    nc.vector.tensor_copy(bias_bc[:, DC * dc:DC * (dc + 1)], pb[:])

            # DRAM staging for the per-chunk ReduceScatter pipeline
            rs_in = [dramp.tile([CHQ, D_MODEL], bf16, tag=f"rs_in{q}",
                                name=f"rs_in{q}") for q in range(NQC)]
            rs_out = [dramp.tile([KT, D_MODEL], bf16, tag=f"rs_out{q}",
                                 name=f"rs_out{q}") for q in range(NQC)]
            cc_wi = dramp.tile([1, 64], bf16, tag="cc_wi", name="cc_wi")
            cc_wo = dramp.tile([1, 16], bf16, tag="cc_wo", name="cc_wo")

            with tc.tile_pool(name="qkvt", bufs=1) as qkvtp, \
                 tc.tile_pool(name="vnat", bufs=1) as vnatp, \
                 tc.tile_pool(name="xT", bufs=1) as xTp, \
                 tc.tile_pool(name="wr", bufs=1) as wrp, \
                 tc.tile_pool(name="wo", bufs=1) as wop, \
                 tc.tile_pool(name="wld", bufs=2) as wldp, \
                 tc.tile_pool(name="za", bufs=1) as zap, \
                 tc.tile_pool(name="pstage", bufs=9) as pstp, \
                 tc.tile_pool(name="rcp", bufs=1) as rcpp, \
                 tc.tile_pool(name="po", bufs=4) as pop, \
                 tc.tile_pool(name="fin", bufs=2) as finp, \
                 tc.tile_pool(name="psPP", bufs=2, space="PSUM") as psPP, \
                 tc.tile_pool(name="psZ", bufs=1, space="PSUM") as psZ, \
                 tc.tile_pool(name="psSM", bufs=1, space="PSUM") as psSM:

                # ---- PE warm-up: ~4us of back-to-back matmuls (HAM ramp) ----
                for wg in range(100):
                    pw = psSM.tile([128, 64], f32, tag="sm", name="pw")
                    nc.tensor.matmul(pw[:], ident_r[:], ident_r[:, 0:64],
                                     start=True, stop=True)
                # dummy collective: absorb first-collective rendezvous cost
                wsb = constp.tile([1, 64], bf16, tag="wsb")
                nc.vector.memset(wsb[:], 0.0)
                nc.sync.dma_start(out=cc_wi[:], in_=wsb[:])
                nc.gpsimd.collective_compute(
                    "ReduceScatter", mybir.AluOpType.add,
                    replica_groups=[[0, 1, 2, 3], [4, 5, 6, 7]],
                    ins=[cc_wi[:].opt()], outs=[cc_wo[:].opt()])

                qkvT = [qkvtp.tile([128, SEQ], bf16, tag=f"qkvT{i}", name=f"qkvT{i}")
                        for i in range(5)]
                # one vnat tile: per k-tile t, head h at cols 195t+65h(+64=ones)
                vnat = vnatp.tile([128, NT * 65 * HPC], bf16, tag="vnat",
                                  name="vnat")
                vnat3 = vnat[:].rearrange("p (t g c) -> p t g c", g=HPC, c=65)
                xT = [xTp.tile([128, SEQ], bf16, tag=f"xT{d}", name=f"xT{d}")
                      for d in range(ND)]
                woA = wop.tile([128, D_MODEL], bf16, tag="woA", name="woA")
                woC = wop.tile([64, D_MODEL], bf16, tag="woC", name="woC")
                zA = {h: zap.tile([64, SEQ], bf16, tag=f"zA{h}", name=f"zA{h}")
                      for h in (1, 2)}
                zpair = zap.tile([128, SEQ], bf16, tag="zpair", name="zpair")
                rcbs = []
                for h in range(HPC):
                    rcb = rcpp.tile([128, CHQ], bf16, tag=f"rcb_{h}",
                                    name=f"rcb_{h}")
                    nc.vector.memset(rcb[:], 0.0)
                    rcbs.append(rcb)

                # ---- input DMAs ----
                for d in range(ND):
                    nc.sync.dma_start(out=xT[d][:],
                                      in_=xT_d[128 * d:128 * (d + 1), :])
                for mt in range(5):
                    nc.gpsimd.dma_start(out=bias_sb[:, mt:mt + 1],
                                        in_=bqkv_d[mt:mt + 1, :])
                wr = []
                for d in range(ND):
                    w16 = wldp.tile([128, 576], bf16, tag="w16")
                    m16 = wldp.tile([128, 576], bf16, tag="m16")
                    nc.sync.dma_start(out=w16[:], in_=wqkv_d[128 * d:128 * (d + 1), :])
                    nc.sync.dma_start(out=m16[:], in_=mqkv_d[128 * d:128 * (d + 1), :])
                    wrt = wrp.tile([128, 576], bf16, tag=f"wr{d}", name=f"wr{d}")
                    nc.vector.tensor_mul(wrt[:], w16[:], m16[:])
                    wr.append(wrt)
                woA32 = wldp.tile([128, D_MODEL], bf16, tag="woA32")
                moA32 = wldp.tile([128, D_MODEL], bf16, tag="moA32")
                nc.sync.dma_start(out=woA32[:], in_=wo_d[0:128, :])
                nc.sync.dma_start(out=moA32[:], in_=mo_d[0:128, :])
                nc.vector.tensor_mul(woA[:], woA32[:], moA32[:])
                woC32 = wldp.tile([64, D_MODEL], bf16, tag="woC32")
                moC32 = wldp.tile([64, D_MODEL], bf16, tag="moC32")
                nc.sync.dma_start(out=woC32[:], in_=wo_d[128:192, :])
                nc.sync.dma_start(out=moC32[:], in_=mo_d[128:192, :])
                nc.vector.tensor_mul(woC[:], woC32[:], moC32[:])
                # ones columns of vnat (col 65h+64 is the row-sum column)
                for t in range(NT):
                    nc.vector.tensor_copy(vnat3[:, t, :, 64], ones3[:])

                hacc = []
                for h in range(HPC):
                    qs, ks = SLOT[("q", h)], SLOT[("k", h)]
                    hacc.append((qkvT[qs // 2], 64 * (qs % 2),
                                 qkvT[ks // 2], 64 * (ks % 2)))

                # =========== main per-chunk pipeline ===========
                # B(qc)/C(qc) run between norm(qc-1) and E(qc-1) so E's
                # zpair shift-DMA has time to land; z-matmuls trail their
                # exp by one pr-pair so the PE never head-of-line blocks.
                def phase_B(qc):
                    for mt in range(5):
                        M = 128 if mt < 4 else 64
                        ps = psPP.tile([128, 2 * CHQ], f32, tag="pp", name="ps")
                        for d in range(ND):
                            nc.tensor.matmul(
                                ps[0:M, 0:CHQ], wr[d][:, 128 * mt:128 * mt + M],
                                xT[d][:, CHQ * qc:CHQ * (qc + 1)],
                                start=(d == 0), stop=(d == ND - 1))
                        nc.vector.tensor_scalar_add(
                            qkvT[mt][0:M, CHQ * qc:CHQ * (qc + 1)],
                            ps[0:M, 0:CHQ], bias_sb[0:M, mt:mt + 1])

                def phase_C(qc):
                    for h in range(HPC):
                        sl = SLOT[("v", h)]
                        base = 64 * (sl % 2)
                        vsrc = qkvT[sl // 2][base:base + 64, :]
                        idb = ident_r[base:base + 64, base:base + 64]
                        pv4 = psSM.tile([128, 4 * 64], f32, tag="sm", name="pv4")
                        for i, t in enumerate(range(NKQ * qc, NKQ * (qc + 1))):
                            nc.tensor.matmul(pv4[:, 64 * i:64 * (i + 1)],
                                             vsrc[:, KT * t:KT * (t + 1)],
                                             idb, start=True, stop=True)
                        nc.vector.tensor_copy(
                            vnat3[:, NKQ * qc:NKQ * (qc + 1), h, 0:64], pv4[:])

                def phase_E(qc):
                    for sub in range(NKQ):
                        qa = CHQ * qc + KT * sub
                        for dc in range(2):
                            pe = psPP.tile([128, DC], f32, tag="pp", name="pe")
                            nc.tensor.matmul(
                                pe[:], zpair[:, qa:qa + KT],
                                woA[:, DC * dc:DC * (dc + 1)],
                                start=True, stop=False)
                            nc.tensor.matmul(
                                pe[:], zA[2][:, qa:qa + KT],
                                woC[:, DC * dc:DC * (dc + 1)],
                                start=False, stop=True)
                            po = pop.tile([128, DC], bf16, tag="po")
                            nc.vector.tensor_copy(po[:], pe[:])
                            nc.sync.dma_start(
                                out=rs_in[qc][KT * sub:KT * (sub + 1),
                                              DC * dc:DC * (dc + 1)],
                                in_=po[:])
                    nc.gpsimd.collective_compute(
                        "ReduceScatter", mybir.AluOpType.add,
                        replica_groups=[[0, 1, 2, 3], [4, 5, 6, 7]],
                        ins=[rs_in[qc][:].opt()], outs=[rs_out[qc][:].opt()])
                    fo = finp.tile([KT, D_MODEL], bf16, tag="fo")
                    nc.sync.dma_start(out=fo[:], in_=rs_out[qc][:])
                    oo = finp.tile([KT, D_MODEL], f32, tag="oo")
                    nc.vector.tensor_add(oo[:], fo[:], bias_bc[:])
                    nc.sync.dma_start(
                        out=out_d[KT * qc:KT * (qc + 1), :], in_=oo[:])

                phase_B(0)
                phase_C(0)
                for qc in range(NQC):
                    # ---- D: causal attention for query chunk qc ----
                    zps = [psZ.tile([65, CHQ], f32, tag=f"zps{h}", name=f"zps{h}")
                           for h in range(HPC)]
                    nkt = NKQ * qc + NKQ

                    def colo(kt, _qc=qc):
                        return (kt - NKQ * _qc) * KT if kt >= NKQ * _qc else 0

                    def z_mms(pr):
                        k0, k1 = 2 * pr, 2 * pr + 1
                        lo0, lo1 = colo(k0), colo(k1)
                        for h in range(HPC):
                            P = pend[pr][h]
                            for j, (kt, lo) in enumerate([(k0, lo0), (k1, lo1)]):
                                if kt >= NKQ * qc:
                                    nc.vector.tensor_mul(
                                        P[:, CHQ * j + lo:CHQ * j + lo + KT],
                                        P[:, CHQ * j + lo:CHQ * j + lo + KT],
                                        tri[:])
                                nc.tensor.matmul(
                                    zps[h][:, lo:],
                                    vnat3[:, kt, h, :],
                                    P[:, CHQ * j + lo:CHQ * (j + 1)],
                                    start=(kt == 0), stop=(kt == nkt - 1))

                    pend = {}
                    for pr in range(nkt // 2):
                        k0, k1 = 2 * pr, 2 * pr + 1
                        lo0, lo1 = colo(k0), colo(k1)
                        pps = [psPP.tile([128, 2 * CHQ], f32, tag="pp",
                                         name=f"pp{h}") for h in range(HPC)]
                        # j1 writes its full span so exp reads only fresh PSUM
                        for j, (kt, lo) in enumerate([(k0, lo0), (k1, 0)]):
                            for h in range(HPC):
                                qT, qb, kT_, kb = hacc[h]
                                nc.tensor.matmul(
                                    pps[h][:, CHQ * j + lo:CHQ * (j + 1)],
                                    kT_[kb:kb + 64, KT * kt:KT * (kt + 1)],
                                    qT[qb:qb + 64, CHQ * qc + lo:CHQ * (qc + 1)],
                                    start=True, stop=True)
                        Ps = []
                        for h in range(HPC):
                            P = pstp.tile([128, 2 * CHQ], bf16, tag="P")
                            nc.scalar.activation(P[:, lo0:], pps[h][:, lo0:],
                                                 FT.Exp, scale=SCALE)
                            Ps.append(P)
                        pend[pr] = Ps
                        if pr > 0:
                            z_mms(pr - 1)
                    z_mms(nkt // 2 - 1)
                    # normalize z by 1/rowsum (row 64); broadcast via 64-row
                    # ones-matmul against zeroed rhs whose row 64 = 1/sum
                    s65s = []
                    for h in range(HPC):
                        s65 = rcpp.tile([65, CHQ], f32, tag=f"s65_{h}", name="s65")
                        nc.vector.tensor_copy(s65[:], zps[h][:])
                        s65s.append(s65)
                    for h in range(HPC):
                        rc65 = rcpp.tile([65, CHQ], f32, tag=f"rc_{h}", name="rc65")
                        sc65 = rcpp.tile([65, CHQ], f32, tag="sc", name="sc65")
                        # custom-DVE recip misbehaves on partial-base APs; run
                        # full-tile at base 0
                        nc.vector.reciprocal_approx_accurate(
                            out=rc65[:], in_=s65s[h][:], scratch=sc65[:])
                        nc.vector.tensor_copy(rcbs[h][64:65, :], rc65[64:65, :])
                        bc = psSM.tile([128, CHQ], f32, tag="sm", name="bc")
                        nc.tensor.matmul(bc[:], ones128b[64:128, 0:128],
                                         rcbs[h][64:128, :], start=True, stop=True)
                        if h == 0:
                            nc.vector.tensor_mul(
                                zpair[0:64, CHQ * qc:CHQ * (qc + 1)],
                                s65s[0][0:64, :], bc[0:64, :])
                        else:
                            nc.vector.tensor_mul(
                                zA[h][:, CHQ * qc:CHQ * (qc + 1)],
                                s65s[h][0:64, :], bc[0:64, :])
                    nc.sync.dma_start(
                        out=zpair[64:128, CHQ * qc:CHQ * (qc + 1)],
                        in_=zA[1][:, CHQ * qc:CHQ * (qc + 1)])
                    phase_E(qc)
                    if qc + 1 < NQC:
                        phase_B(qc + 1)
                        phase_C(qc + 1)
    nc.compile()
    return nc


def _get_nc():
    global _BUILT
    if _BUILT is None:
        _BUILT = _build()
    return _BUILT


def _make_in_maps(inputs):
    import concourse.mybir as mybir
    f = np.float32
    bf = mybir.dt.np(mybir.dt.bfloat16)
    x = np.asarray(inputs["normalized_resid_pre"], f)
    W = {"q": np.asarray(inputs["W_Q"], f), "k": np.asarray(inputs["W_K"], f),
         "v": np.asarray(inputs["W_V"], f)}
    Mm = {"q": np.asarray(inputs["mask_W_Q"], f), "k": np.asarray(inputs["mask_W_K"], f),
          "v": np.asarray(inputs["mask_W_V"], f)}
    B = {"q": np.asarray(inputs["b_Q"], f), "k": np.asarray(inputs["b_K"], f),
         "v": np.asarray(inputs["b_V"], f)}
    wo_full = np.asarray(inputs["W_O"], f)      # [12, 64, 768]
    mo_full = np.asarray(inputs["mask_W_O"], f)
    bo = np.asarray(inputs["b_O"], f).reshape(1, D_MODEL)

    xT = [np.ascontiguousarray(x[b].T).astype(bf) for b in range(BATCH)]

    in_maps = []
    for c in range(N_CORES):
        b, g = divmod(c, GROUP)
        heads = [HPC * g + i for i in range(HPC)]
        wqkv = np.zeros((D_MODEL, 576), f)
        mqkv = np.zeros((D_MODEL, 576), f)
        bqkv = np.zeros((5, 128), f)
        for s, (mat, hh) in enumerate(SLOTS):
            gh = heads[hh]
            wqkv[:, 64 * s:64 * (s + 1)] = W[mat][gh]
            mqkv[:, 64 * s:64 * (s + 1)] = Mm[mat][gh]
            bqkv[s // 2, 64 * (s % 2):64 * (s % 2) + 64] = B[mat][gh]
        wo = np.ascontiguousarray(
            wo_full[heads].reshape(HPC * D_HEAD, D_MODEL)).astype(bf)
        mo = np.ascontiguousarray(
            mo_full[heads].reshape(HPC * D_HEAD, D_MODEL)).astype(bf)
        in_maps.append({
            "xT": xT[b],
            "wqkv": wqkv.astype(bf), "mqkv": mqkv.astype(bf), "bqkv": bqkv,
            "wo": wo, "mo": mo, "bo": bo,
        })
    return in_maps


def _run(inputs, trace=False):
    from concourse.bass_utils import run_bass_kernel_spmd
    nc = _get_nc()
    res = run_bass_kernel_spmd(nc, _make_in_maps(inputs),
                               core_ids=list(range(N_CORES)), trace=trace)
    out = np.empty((BATCH, SEQ, D_MODEL), np.float32)
    for c in range(N_CORES):
        b, r = divmod(c, GROUP)
        o = res.results[c]["out"]  # [512, 768]: 4 chunks of 128 rows
        for qc in range(NQC):
            out[b, CHQ * qc + KT * r:CHQ * qc + KT * (r + 1), :] = \
                o[KT * qc:KT * (qc + 1)]
    return out, res


def kernel(**inputs):
    out, _ = _run(inputs, trace=False)
    return out
